# revision 1
# baseline (speedup 1.0000x reference)
"""Trainium2 Bass kernel for a gated LoRA adapter layer (MoE-style routing).

Computes, for x:(8,2048,4096) f32, type_weight:(8,2048) f32,
lora_A:(4096,64) f32, lora_B:(64,4096) f32:

    out = type_weight[..., None] * ((x @ lora_A) @ lora_B) * 2.0

Sharding: data-parallel over the batch axis — core i gets x[i], type_weight[i];
lora_A / lora_B are replicated. Each core:

  1. SWDGE cast-DMA loads x stripes [128, 4096] HBM f32 -> SBUF bf16.
  2. One XBAR DMA-transpose per stripe produces xT[p, dt, s] = x[s, dt*128+p]
     (the contraction dim d lands on partitions, required by the PE).
  3. mm1 (bf16): t.T[64, 512] = sum_dt A[dt].T-block @ xT[:, dt, :]  (PSUM f32)
  4. One DVE op fuses PSUM->SBUF copy with the (2.0 * type_weight) scaling
     (type_weight replicated across the 64 R-partitions once per core).
  5. mm2 (float32r, full-rate): out[128, 512] = tT.T-slice @ B chunks.
  6. PSUM->SBUF copy + DMA store of each output tile.
"""

import numpy as np

B_CORES = 8
S = 2048
D = 4096
R = 64
LORA_SCALING = 128.0 / 64.0

S_CHUNK = 512  # moving free dim for mm1 / row block for mm2 group
N_SCHUNKS = S // S_CHUNK  # 4
N_DT = D // 128  # 32 d-tiles
N_DC = D // 512  # 8 output column chunks

_CACHE = {}


def _build_bass():
    import concourse.tile as tile
    from concourse import bacc, mybir

    nc = bacc.Bacc(
        "TRN2",
        debug=False,
        enable_asserts=False,
        target_bir_lowering=False,
        num_devices=B_CORES,
    )

    x_d = nc.dram_tensor("x", [S, D], mybir.dt.float32, kind="ExternalInput").ap()
    tw_d = nc.dram_tensor("tw", [1, S], mybir.dt.float32, kind="ExternalInput").ap()
    a_d = nc.dram_tensor("lora_a", [D, R], mybir.dt.float32, kind="ExternalInput").ap()
    b_d = nc.dram_tensor(
        "lora_b", [R, D], mybir.dt.float32r, kind="ExternalInput"
    ).ap()
    out_d = nc.dram_tensor("out", [S, D], mybir.dt.float32, kind="ExternalOutput").ap()

    f32 = mybir.dt.float32
    f32r = mybir.dt.float32r
    bf16 = mybir.dt.bfloat16

    with tile.TileContext(nc) as tc:
        with (
            tc.tile_pool(name="consts", bufs=1) as consts,
            tc.tile_pool(name="xin", bufs=6) as xin,
            tc.tile_pool(name="xt", bufs=2) as xtp,
            tc.tile_pool(name="tt", bufs=2) as ttp,
            tc.tile_pool(name="osb", bufs=4) as osb,
            tc.tile_pool(name="ps_t", bufs=2, space="PSUM") as ps_t,
            tc.tile_pool(name="ps_o", bufs=4, space="PSUM") as ps_o,
        ):
            # Replicated weights. A: [D, R] -> [p, dt, r] with d = dt*128 + p
            # (matches the XBAR transpose layout of x).
            a_sb = consts.tile([128, N_DT, R], bf16)
            nc.gpsimd.dma_start(
                a_sb[:], a_d.rearrange("(dt p) r -> p dt r", p=128)
            )
            b_sb = consts.tile([R, D], f32r)
            nc.sync.dma_start(b_sb[:], b_d)

            # type_weight * 2.0 replicated across the R partitions:
            # tw_rep[r, s] = 2 * tw[s], built with a K=1 matmul against a
            # constant-2.0 column.
            two_sb = consts.tile([1, R], f32)
            nc.any.memset(two_sb[:], LORA_SCALING)
            tw_sb = consts.tile([1, S], f32)
            nc.sync.dma_start(tw_sb[:], tw_d)
            tw_rep = consts.tile([R, S], f32)
            for sc in range(N_SCHUNKS):
                ps_tw = ps_t.tile([R, S_CHUNK], f32)
                nc.tensor.matmul(
                    ps_tw[:],
                    lhsT=two_sb[:],
                    rhs=tw_sb[:, sc * S_CHUNK : (sc + 1) * S_CHUNK],
                    start=True,
                    stop=True,
                )
                nc.vector.tensor_copy(
                    tw_rep[:, sc * S_CHUNK : (sc + 1) * S_CHUNK], ps_tw[:]
                )

            for sc in range(N_SCHUNKS):
                s0 = sc * S_CHUNK
                # x stripes: HBM f32 -> SBUF bf16 (cast during SWDGE DMA),
                # then XBAR transpose to put d on partitions.
                xt = xtp.tile([128, N_DT, S_CHUNK], bf16)
                for k in range(S_CHUNK // 128):
                    x_sb = xin.tile([128, D], bf16)
                    nc.gpsimd.dma_start(
                        x_sb[:], x_d[s0 + k * 128 : s0 + (k + 1) * 128, :]
                    )
                    nc.sync.dma_start_transpose(
                        xt[:, :, k * 128 : (k + 1) * 128], x_sb[:]
                    )

                # mm1: t.T[r, s] accumulated over the 32 d-tiles.
                ps_tt = ps_t.tile([R, S_CHUNK], f32)
                for dt in range(N_DT):
                    nc.tensor.matmul(
                        ps_tt[:],
                        lhsT=a_sb[:, dt, :],
                        rhs=xt[:, dt, :],
                        start=(dt == 0),
                        stop=(dt == N_DT - 1),
                    )

                # Fused PSUM->SBUF + gate scaling: tT = t.T * (2 * tw).
                tt_sb = ttp.tile([R, S_CHUNK], f32r)
                nc.vector.tensor_tensor(
                    tt_sb[:],
                    ps_tt[:],
                    tw_rep[:, s0 : s0 + S_CHUNK],
                    mybir.AluOpType.mult,
                )

                # mm2: out[s, d] = tT.T @ B, in [128, 512] tiles.
                for st in range(S_CHUNK // 128):
                    for dc in range(N_DC):
                        ps_out = ps_o.tile([128, 512], f32)
                        nc.tensor.matmul(
                            ps_out[:],
                            lhsT=tt_sb[:, st * 128 : (st + 1) * 128],
                            rhs=b_sb[:, dc * 512 : (dc + 1) * 512],
                            start=True,
                            stop=True,
                        )
                        o_sb = osb.tile([128, 512], f32)
                        nc.any.tensor_copy(out=o_sb[:], in_=ps_out[:])
                        nc.sync.dma_start(
                            out_d[
                                s0 + st * 128 : s0 + (st + 1) * 128,
                                dc * 512 : (dc + 1) * 512,
                            ],
                            o_sb[:],
                        )

    nc.compile()
    return nc


def get_bass():
    if "nc" not in _CACHE:
        _CACHE["nc"] = _build_bass()
    return _CACHE["nc"]


def make_in_maps(x, type_weight, lora_A, lora_B):
    x = np.asarray(x, dtype=np.float32)
    tw = np.asarray(type_weight, dtype=np.float32)
    a = np.ascontiguousarray(np.asarray(lora_A, dtype=np.float32))
    b = np.ascontiguousarray(np.asarray(lora_B, dtype=np.float32))
    return [
        {
            "x": np.ascontiguousarray(x[i]),
            "tw": np.ascontiguousarray(tw[i]).reshape(1, S),
            "lora_a": a,
            "lora_b": b,
        }
        for i in range(B_CORES)
    ]


def kernel(x, type_weight, lora_A, lora_B):
    from concourse.bass_utils import run_bass_kernel_spmd

    nc = get_bass()
    in_maps = make_in_maps(x, type_weight, lora_A, lora_B)
    res = run_bass_kernel_spmd(nc, in_maps, list(range(B_CORES)))
    out = np.stack([res.results[i]["out"] for i in range(B_CORES)], axis=0)
    return out.astype(np.float32, copy=False)


if __name__ == "__main__":
    nc = get_bass()
    print("built + compiled ok")


# revision 8
# speedup vs baseline: 1.3577x; 1.3577x over previous
"""Trainium2 Bass kernel for a gated LoRA adapter layer (MoE-style routing).

Computes, for x:(8,2048,4096) f32, type_weight:(8,2048) f32,
lora_A:(4096,64) f32, lora_B:(64,4096) f32:

    out = type_weight[..., None] * ((x @ lora_A) @ lora_B) * 2.0

Sharding: data-parallel over the batch axis — core i gets x[i], type_weight[i];
lora_A / lora_B are replicated. Each core:

  1. SWDGE cast-DMA loads x stripes [128, 4096] HBM f32 -> SBUF bf16.
  2. TensorE transposes each [128, 128] block (bf16, via identity) so the
     contraction dim d lands on partitions; DVE copies PSUM -> SBUF.
  3. mm1 (bf16): t.T[64, 512] = sum_dt A[dt]-block.T @ xT[:, dt, :] (PSUM f32)
  4. One DVE op fuses the PSUM->SBUF copy with the (2.0 * type_weight)
     scaling (type_weight replicated across the 64 R-partitions once).
  5. mm2 (bf16): out[128, 512] = tT-slice.T @ B chunks.
  6. PSUM->SBUF copy (ScalarE) + DMA store of each output tile.
"""

import numpy as np

B_CORES = 8
S = 2048
D = 4096
R = 64
LORA_SCALING = 128.0 / 64.0

S_CHUNK = 512  # moving free dim for mm1 / row block for mm2 group
N_SCHUNKS = S // S_CHUNK  # 4
N_DT = D // 128  # 32 d-tiles
N_DC = D // 512  # 8 output column chunks

_CACHE = {}

# Build-time tuning knobs (read once at _build_bass time).
OPTS = {
    "xin_bufs": 10,
    "xt_bufs": 2,
    "ps_x_bufs": 3,
    "ps_o_bufs": 3,
    "osb_bufs": 16,
    "xt_copy": "vector",  # vector | any | alt (alternate vector/scalar)
    "out_copy": "scalar",  # any | vector | scalar
    "big_store": False,
}


def _build_bass():
    import concourse.tile as tile
    from concourse import bacc, mybir
    from concourse.masks import make_identity

    nc = bacc.Bacc(
        "TRN2",
        debug=False,
        enable_asserts=False,
        target_bir_lowering=False,
        num_devices=B_CORES,
    )

    x_d = nc.dram_tensor("x", [S, D], mybir.dt.float32, kind="ExternalInput").ap()
    tw_d = nc.dram_tensor("tw", [1, S], mybir.dt.float32, kind="ExternalInput").ap()
    a_d = nc.dram_tensor("lora_a", [D, R], mybir.dt.float32, kind="ExternalInput").ap()
    b_d = nc.dram_tensor("lora_b", [R, D], mybir.dt.float32, kind="ExternalInput").ap()
    out_d = nc.dram_tensor("out", [S, D], mybir.dt.float32, kind="ExternalOutput").ap()

    f32 = mybir.dt.float32
    bf16 = mybir.dt.bfloat16

    with tile.TileContext(nc) as tc:
        with (
            tc.tile_pool(name="consts", bufs=1) as consts,
            tc.tile_pool(name="xin", bufs=OPTS["xin_bufs"]) as xin,
            tc.tile_pool(name="xt", bufs=OPTS["xt_bufs"]) as xtp,
            tc.tile_pool(name="tt", bufs=2) as ttp,
            tc.tile_pool(name="osb", bufs=OPTS["osb_bufs"]) as osb,
            tc.tile_pool(name="ps_x", bufs=OPTS["ps_x_bufs"], space="PSUM") as ps_x,
            tc.tile_pool(name="ps_t", bufs=2, space="PSUM") as ps_t,
            tc.tile_pool(name="ps_o", bufs=OPTS["ps_o_bufs"], space="PSUM") as ps_o,
        ):
            ident = consts.tile([128, 128], bf16)
            make_identity(nc, ident[:])

            # Replicated weights. A: [D, R] -> [p, dt, r] with d = dt*128 + p.
            a_sb = consts.tile([128, N_DT, R], bf16)
            nc.gpsimd.dma_start(a_sb[:], a_d.rearrange("(dt p) r -> p dt r", p=128))
            b_sb = consts.tile([R, D], bf16)
            nc.gpsimd.dma_start(b_sb[:], b_d)

            # type_weight * 2.0 replicated across the R partitions:
            # tw_rep[r, s] = 2 * tw[s], built with a K=1 matmul against a
            # constant-2.0 column.
            two_sb = consts.tile([1, R], f32)
            nc.any.memset(two_sb[:], LORA_SCALING)
            tw_sb = consts.tile([1, S], f32)
            nc.sync.dma_start(tw_sb[:], tw_d)
            tw_rep = consts.tile([R, S], f32)
            for sc in range(N_SCHUNKS):
                ps_tw = ps_t.tile([R, S_CHUNK], f32, tag="t")
                nc.tensor.matmul(
                    ps_tw[:],
                    lhsT=two_sb[:],
                    rhs=tw_sb[:, sc * S_CHUNK : (sc + 1) * S_CHUNK],
                    start=True,
                    stop=True,
                )
                nc.vector.tensor_copy(
                    tw_rep[:, sc * S_CHUNK : (sc + 1) * S_CHUNK], ps_tw[:]
                )

            for sc in range(N_SCHUNKS):
                s0 = sc * S_CHUNK
                # x stripes: HBM f32 -> SBUF bf16 (cast during SWDGE DMA),
                # then TensorE 128x128 transposes put d on partitions.
                xt = xtp.tile([128, N_DT, S_CHUNK], bf16)
                for k in range(S_CHUNK // 128):
                    x_sb = xin.tile([128, D], bf16)
                    nc.gpsimd.dma_start(
                        x_sb[:], x_d[s0 + k * 128 : s0 + (k + 1) * 128, :]
                    )
                    for dt in range(N_DT):
                        psx = ps_x.tile([128, 128], bf16)
                        nc.tensor.transpose(
                            psx[:], x_sb[:, dt * 128 : (dt + 1) * 128], ident[:]
                        )
                        if OPTS["xt_copy"] == "vector" or (
                            OPTS["xt_copy"] == "alt" and dt % 2 == 0
                        ):
                            nc.vector.tensor_copy(
                                xt[:, dt, k * 128 : (k + 1) * 128], psx[:]
                            )
                        elif OPTS["xt_copy"] == "any":
                            nc.any.tensor_copy(
                                out=xt[:, dt, k * 128 : (k + 1) * 128], in_=psx[:]
                            )
                        else:
                            nc.scalar.copy(
                                xt[:, dt, k * 128 : (k + 1) * 128], psx[:]
                            )

                # mm1: t.T[r, s] accumulated over the 32 d-tiles.
                ps_tt = ps_t.tile([R, S_CHUNK], f32, tag="t")
                for dt in range(N_DT):
                    nc.tensor.matmul(
                        ps_tt[:],
                        lhsT=a_sb[:, dt, :],
                        rhs=xt[:, dt, :],
                        start=(dt == 0),
                        stop=(dt == N_DT - 1),
                    )

                # Fused PSUM->SBUF + gate scaling: tT = t.T * (2 * tw).
                tt_sb = ttp.tile([R, S_CHUNK], bf16)
                nc.vector.tensor_tensor(
                    tt_sb[:],
                    ps_tt[:],
                    tw_rep[:, s0 : s0 + S_CHUNK],
                    mybir.AluOpType.mult,
                )

                # mm2: out[s, d] = tT.T @ B, in [128, 512] tiles.
                for st in range(S_CHUNK // 128):
                    for dc in range(N_DC):
                        ps_out = ps_o.tile([128, 512], f32)
                        nc.tensor.matmul(
                            ps_out[:],
                            lhsT=tt_sb[:, st * 128 : (st + 1) * 128],
                            rhs=b_sb[:, dc * 512 : (dc + 1) * 512],
                            start=True,
                            stop=True,
                        )
                        if OPTS["big_store"]:
                            if dc == 0:
                                o_row = osb.tile([128, D], f32, tag="orow")
                            o_sb = o_row[:, dc * 512 : (dc + 1) * 512]
                        else:
                            o_sb = osb.tile([128, 512], f32)
                        if OPTS["out_copy"] == "any":
                            nc.any.tensor_copy(out=o_sb[:], in_=ps_out[:])
                        elif OPTS["out_copy"] == "vector":
                            nc.vector.tensor_copy(o_sb[:], ps_out[:])
                        else:
                            nc.scalar.copy(o_sb[:], ps_out[:])
                        if OPTS["big_store"]:
                            if dc == N_DC - 1:
                                nc.sync.dma_start(
                                    out_d[s0 + st * 128 : s0 + (st + 1) * 128, :],
                                    o_row[:],
                                )
                        else:
                            nc.sync.dma_start(
                                out_d[
                                    s0 + st * 128 : s0 + (st + 1) * 128,
                                    dc * 512 : (dc + 1) * 512,
                                ],
                                o_sb[:],
                            )

    nc.compile()
    return nc


def get_bass():
    if "nc" not in _CACHE:
        _CACHE["nc"] = _build_bass()
    return _CACHE["nc"]


def make_in_maps(x, type_weight, lora_A, lora_B):
    x = np.asarray(x, dtype=np.float32)
    tw = np.asarray(type_weight, dtype=np.float32)
    a = np.ascontiguousarray(np.asarray(lora_A, dtype=np.float32))
    b = np.ascontiguousarray(np.asarray(lora_B, dtype=np.float32))
    return [
        {
            "x": np.ascontiguousarray(x[i]),
            "tw": np.ascontiguousarray(tw[i]).reshape(1, S),
            "lora_a": a,
            "lora_b": b,
        }
        for i in range(B_CORES)
    ]


def kernel(x, type_weight, lora_A, lora_B):
    from concourse.bass_utils import run_bass_kernel_spmd

    nc = get_bass()
    in_maps = make_in_maps(x, type_weight, lora_A, lora_B)
    res = run_bass_kernel_spmd(nc, in_maps, list(range(B_CORES)))
    out = np.stack([res.results[i]["out"] for i in range(B_CORES)], axis=0)
    return out.astype(np.float32, copy=False)


if __name__ == "__main__":
    nc = get_bass()
    print("built + compiled ok")
